# revision 18
# baseline (speedup 1.0000x reference)
"""TF-IDF document model (histogram_binning) on 8 TRN2 NeuronCores.

Data-parallel over batch: 64 rows per core. Per row, the tf histogram over
vocab V=50257 is computed as a radix one-hot matmul on the PE:
v = hi*394 + lo, hi in [0,128), lo in [0,394).

Key structure (vs. the naive per-chunk one-hot kernel):
  - Host sorts each row's tokens by lo. Chunk c (sorted positions
    [128c,128c+128)) then covers a narrow static lo-window [Q[c], Q[c]+W[c]),
    so each accumulating matmul streams only ~100 output columns instead of
    394. (Windows are validated against the input; a data-derived build is
    used as fallback.)
  - All 8 hi one-hots of a row are built by ONE DVE tensor_tensor is_equal
    with a broadcast access pattern (in0 = hif row chunk broadcast along an
    h-major axis), hitting the 2x DVE mode: A_int[p, h*8+c] = (hif[p,c]==h).
    The matmul lhsT reads the per-chunk one-hot via a strided AP.
  - The per-token idf value (host gather idf[x], like the baseline's host
    divmod) rides the lo one-hot build as the tensor_scalar op1 multiplier,
    so no separate (B,V)-sized tf*idf multiply pass exists.
  - The lo one-hot builds are split between the DVE and the otherwise idle
    GPSIMD (Pool) engine (alternating 4/3 per row to balance).
  - PSUM is cleared by a K=1 zero matmul, the 8 windowed matmuls accumulate,
    and the ACT engine's PSUM->SBUF copy applies the per-row 1/n scale and
    converts to fp16 (halving the output DMA). Host upcasts to fp32.
  - Inputs are packed into two large DMAs (the cost of a DMA dispatch is
    dominated by fixed HWDGE/SEQ overheads).
"""
import numpy as np

import concourse.bacc as bacc
import concourse.mybir as mybir
from concourse import bass_utils
from concourse.tile import TileContext

B, S, V = 512, 1024, 50257
NC = 8
BL = B // NC          # 64 rows per core
HI, LO = 128, 394     # radix split: v = hi*LO + lo
VP = HI * LO          # 50432 padded vocab
CH = S // 128         # 8 sorted 128-token chunks per row
GROUP = 2             # rows per output DMA

# static lo-windows per sorted chunk (observed data bounds +-4; the host
# prep asserts every token falls inside its window, kernel() falls back to
# a data-derived build if violated)
QS = [0, 33, 76, 124, 174, 220, 273, 326]
WS = [68, 88, 96, 96, 96, 96, 88, 68]

_cache = {}


def _dve_b_chunks(r):
    """Which chunks' lo-builds run on the DVE for row r (rest on Pool)."""
    return (2, 3, 4, 5) if r % 4 < 3 else (3, 4, 5)


def _build(repeat: int = 0, feat: str = "full", qs=None, ws=None):
    QS, WS = (qs or globals()["QS"]), (ws or globals()["WS"])
    WMAX = max(WS)
    nc = bacc.Bacc(
        "TRN2",
        target_bir_lowering=False,
        debug=False,
        enable_asserts=False,
        num_devices=NC,
    )
    ncols = BL * CH
    # packed inputs: ONE fp16 DMA k16 = iotar(HI) ++ iotaw(WMAX) ++ hif(ncols)
    # (iotar is [128,HI] and broadcast along the chunk axis in the TT in1),
    # and pk32 = lof(ncols) ++ idfv(ncols) ++ onesc(1)
    K16 = HI * CH + WMAX + ncols
    P32 = 2 * ncols + 1
    ck16_t = nc.dram_tensor("ck16", [128, K16], mybir.dt.float16, kind="ExternalInput")
    pk32_t = nc.dram_tensor("pk32", [128, P32], mybir.dt.float32, kind="ExternalInput")
    z16_t = nc.dram_tensor("z16", [1, 128 + LO], mybir.dt.float16, kind="ExternalInput")
    ones32_t = nc.dram_tensor("ones32", [1, 128], mybir.dt.float32, kind="ExternalInput")
    # transposed layout: out[p, r*LO+f] = row r, vocab p*LO+f (host unshuffles)
    out_t = nc.dram_tensor("out", [128, BL * LO], mybir.dt.float16, kind="ExternalOutput")
    ovg = out_t.ap().rearrange("p (g c) -> g p c", g=BL // GROUP)

    AF = mybir.ActivationFunctionType
    OP = mybir.AluOpType

    with TileContext(nc) as tc:
        with (
            tc.tile_pool(name="const", bufs=1) as cpool,
            tc.tile_pool(name="aall", bufs=10) as apool,
            tc.tile_pool(name="bt", bufs=64) as bpool,
            tc.tile_pool(name="tt", bufs=6) as tpool,
            tc.tile_pool(name="small", bufs=2) as spool,
            tc.tile_pool(name="ps", bufs=6, space="PSUM") as pspool,
            tc.tile_pool(name="ps2", bufs=1, space="PSUM") as ps2pool,
        ):
            # two parallel input queues: sync carries everything fp16 (one
            # DMA), scalar carries the fp32 pack (head latency)
            ck16 = cpool.tile([128, K16], mybir.dt.float16, tag="ck16")
            nc.sync.dma_start(out=ck16[:], in_=ck16_t.ap())
            pk32 = cpool.tile([128, P32], mybir.dt.float32, tag="pk32")
            nc.scalar.dma_start(out=pk32[:], in_=pk32_t.ap())
            z16 = cpool.tile([1, 128 + LO], mybir.dt.float16, tag="z16")
            nc.scalar.dma_start(out=z16[:], in_=z16_t.ap())
            ones32 = cpool.tile([1, 128], mybir.dt.float32, tag="ones32")
            nc.scalar.dma_start(out=ones32[:], in_=ones32_t.ap())

            iotar = ck16[:, 0 : HI * CH]
            iotaw = ck16[:, HI * CH : HI * CH + WMAX]
            hif = ck16[:, HI * CH + WMAX : HI * CH + WMAX + ncols]
            lof = pk32[:, 0:ncols]
            idfv = pk32[:, ncols : 2 * ncols]
            onesc = pk32[:, 2 * ncols : 2 * ncols + 1]
            zcol = z16[:, 0:128]
            zrow = z16[:, 128 : 128 + LO]
            onesr = ones32[:, :]

            iotar3 = iotar.rearrange("p (h c) -> p h c", c=CH)

            def main_body(_iv=None):
                # --- per-row 1/n: n_r = sum_t idf[x[r,t]] ---
                n_ps = ps2pool.tile([1, ncols], mybir.dt.float32, tag="nps")
                nc.tensor.matmul(out=n_ps[:], lhsT=onesc, rhs=idfv, start=True, stop=True)
                nsum = spool.tile([1, BL], mybir.dt.float32, tag="nsum")
                nc.vector.tensor_reduce(
                    out=nsum[:],
                    in_=n_ps[:].rearrange("p (r c) -> p r c", c=CH),
                    axis=mybir.AxisListType.X,
                    op=OP.add,
                )
                recip = spool.tile([1, BL], mybir.dt.float32, tag="recip")
                nc.vector.reciprocal(out=recip[:], in_=nsum[:])
                rb_ps = ps2pool.tile([128, BL], mybir.dt.float32, tag="rbps")
                nc.tensor.matmul(out=rb_ps[:], lhsT=onesr, rhs=recip[:], start=True, stop=True)
                rb = spool.tile([128, BL], mybir.dt.float32, tag="rb")
                nc.scalar.activation(out=rb[:], in_=rb_ps[:], func=AF.Copy, scale=1.0)

                ngroups = BL // GROUP
                for g in range(ngroups):
                    # the final group is split into per-row DMAs so the
                    # end-of-kernel drain->DMA chain is as short as possible
                    split_tail = g == ngroups - 1
                    if not split_tail:
                        Tg = tpool.tile([128, GROUP * LO], mybir.dt.float16, tag="Tg")
                    for rr in range(GROUP):
                        r = g * GROUP + rr
                        dve_chunks = _dve_b_chunks(r)
                        # fused hi one-hots: A_int[p, h*CH+c] = (hif[p, r*CH+c] == h)
                        Aall = apool.tile([128, HI * CH], mybir.dt.float16, tag="Aall")
                        hif_exp = hif[:, r * CH : (r + 1) * CH].unsqueeze(1).broadcast_to(
                            [128, HI, CH]
                        )
                        nc.vector.tensor_tensor(
                            out=Aall[:].rearrange("p (h c) -> p h c", c=CH),
                            in0=hif_exp,
                            in1=iotar3,
                            op=OP.is_equal,
                        )
                        Aall3 = Aall[:].rearrange("p (h c) -> p c h", c=CH)

                        C = pspool.tile([128, LO], mybir.dt.float32, tag="C")
                        nc.tensor.matmul(out=C[:], lhsT=zcol, rhs=zrow, start=True, stop=False)
                        for c in range(CH):
                            col = r * CH + c
                            Bt = bpool.tile([128, WMAX], mybir.dt.float16, tag="B")
                            eng = nc.vector if c in dve_chunks else nc.gpsimd
                            eng.tensor_scalar(
                                out=Bt[:, : WS[c]],
                                in0=iotaw[:, : WS[c]],
                                scalar1=lof[:, col : col + 1],
                                scalar2=idfv[:, col : col + 1],
                                op0=OP.is_equal,
                                op1=OP.mult,
                            )
                            nc.tensor.matmul(
                                out=C[:, QS[c] : QS[c] + WS[c]],
                                lhsT=Aall3[:, c, :],
                                rhs=Bt[:, : WS[c]],
                                start=False,
                                stop=(c == CH - 1),
                            )
                        if split_tail:
                            # drain the two final rows on DIFFERENT engines
                            # (ACT + DVE) and dispatch their DMAs on
                            # different queues, shortening the end chain
                            Tr = tpool.tile([128, LO], mybir.dt.float16, tag="Tr")
                            if rr == GROUP - 1:
                                nc.vector.tensor_scalar(
                                    out=Tr[:],
                                    in0=C[:],
                                    scalar1=rb[:, r : r + 1],
                                    scalar2=None,
                                    op0=OP.mult,
                                )
                            else:
                                nc.scalar.activation(
                                    out=Tr[:],
                                    in_=C[:],
                                    func=AF.Copy,
                                    scale=rb[:, r : r + 1],
                                )
                            if feat == "nodma":
                                nc.vector.tensor_copy(out=nsum[:, :1], in_=Tr[:1, :1])
                            else:
                                q = nc.sync if rr == GROUP - 1 else nc.scalar
                                q.dma_start(
                                    out=out_t.ap()[:, r * LO : (r + 1) * LO], in_=Tr[:]
                                )
                        else:
                            nc.scalar.activation(
                                out=Tg[:, rr * LO : (rr + 1) * LO],
                                in_=C[:],
                                func=AF.Copy,
                                scale=rb[:, r : r + 1],
                            )
                    if not split_tail:
                        if feat == "nodma":
                            nc.vector.tensor_copy(out=nsum[:, :1], in_=Tg[:1, :1])
                        else:
                            nc.sync.dma_start(out=ovg[g], in_=Tg[:])

            if repeat:
                tc.For_i_unrolled(0, repeat, 1, main_body, max_unroll=1)
            else:
                main_body()
    nc.compile()
    return nc


def _get_nc():
    if "nc" not in _cache:
        _cache["nc"] = _build()
    return _cache["nc"]


def _fits(lo_s: np.ndarray, qs, ws) -> bool:
    lo_c = lo_s.reshape(B, CH, 128)
    qa = np.asarray(qs, dtype=np.int32)[None, :, None]
    wa = np.asarray(ws, dtype=np.int32)[None, :, None]
    return bool(((lo_c >= qa) & (lo_c < qa + wa)).all())


def _windows_from_data(lo_s: np.ndarray):
    """Data-derived safe windows (used only if the static ones don't fit)."""
    qs, ws = [], []
    lo_c = lo_s.reshape(B, CH, 128)
    for c in range(CH):
        lo_b = max(0, int(lo_c[:, c].min()) - 8)
        hi_b = min(LO, int(lo_c[:, c].max()) + 1 + 8)
        w = (hi_b - lo_b + 3) // 4 * 4
        if lo_b + w > LO:
            lo_b = LO - w
        qs.append(lo_b)
        ws.append(w)
    return qs, ws


def _host_inputs(x: np.ndarray, idf: np.ndarray, qs=None, ws=None):
    """Build per-core input maps from the full inputs."""
    qs, ws = (qs or QS), (ws or WS)
    wmax = max(ws)
    xi = np.asarray(x, dtype=np.int64).astype(np.int32)  # values < 2**31
    idf32 = np.asarray(idf, dtype=np.float32)
    hi_all = (xi // LO).astype(np.int32)
    lo_all = (xi % LO).astype(np.int32)

    # sort each row's tokens by lo so each 128-chunk falls in a narrow window
    order = np.argsort(lo_all, axis=1, kind="stable")
    hi_s = np.take_along_axis(hi_all, order, axis=1)
    lo_s = np.take_along_axis(lo_all, order, axis=1)
    xs = np.take_along_axis(xi, order, axis=1)
    idfv_s = idf32[xs]  # (B, S) fp32, host gather (index prep like hif/lof)

    # per-chunk window-local lo
    qa = np.asarray(qs, dtype=np.int32)
    wa = np.asarray(ws, dtype=np.int32)
    lo_c = lo_s.reshape(B, CH, 128) - qa[None, :, None]
    assert lo_c.min() >= 0 and (lo_c < wa[None, :, None]).all(), "lo window overflow"

    hif = hi_s.astype(np.float16)
    lof = lo_c.reshape(B, S).astype(np.float32)
    idfv = idfv_s.astype(np.float32)

    iotar = np.repeat(np.arange(HI, dtype=np.float16), CH)
    iotaw = np.arange(wmax, dtype=np.float16)
    consts16 = np.concatenate([iotar, iotaw])  # shared across partitions
    z16 = np.zeros((1, 128 + LO), dtype=np.float16)
    ones32 = np.ones((1, 128), dtype=np.float32)

    ncols = BL * CH
    in_maps = []
    for k in range(NC):
        # layout [128, BL*CH]: element [p, r*CH+c] = token (row r, sorted pos c*128+p)
        def lay(a):
            ac = a[k * BL : (k + 1) * BL]
            return np.ascontiguousarray(
                ac.reshape(BL, CH, 128).transpose(2, 0, 1).reshape(128, BL * CH)
            )
        ck16 = np.empty((128, len(consts16) + ncols), dtype=np.float16)
        ck16[:, : len(consts16)] = consts16[None, :]
        ck16[:, len(consts16) :] = lay(hif)
        pk32 = np.empty((128, 2 * ncols + 1), dtype=np.float32)
        pk32[:, :ncols] = lay(lof)
        pk32[:, ncols : 2 * ncols] = lay(idfv)
        pk32[:, 2 * ncols] = 1.0
        in_maps.append({"ck16": ck16, "pk32": pk32, "z16": z16, "ones32": ones32})
    return in_maps


def kernel(x: np.ndarray, idf: np.ndarray) -> np.ndarray:
    # check the static windows against this input; fall back to data-derived
    # windows (fresh build) if they don't fit
    xi = np.asarray(x, dtype=np.int64).astype(np.int32)
    lo_s = np.sort((xi % LO).astype(np.int32), axis=1)
    if _fits(lo_s, QS, WS):
        nc = _get_nc()
        in_maps = _host_inputs(x, idf)
    else:
        qs, ws = _windows_from_data(lo_s)
        key = ("dyn", tuple(qs), tuple(ws))
        if key not in _cache:
            _cache[key] = _build(qs=qs, ws=ws)
        nc = _cache[key]
        in_maps = _host_inputs(x, idf, qs, ws)
    res = bass_utils.run_bass_kernel_spmd(nc, in_maps, core_ids=list(range(NC)))
    outs = []
    for r in res.results:
        a = r["out"].reshape(128, BL, LO).transpose(1, 0, 2).reshape(BL, VP)
        outs.append(a[:, :V].astype(np.float32))
    return np.concatenate(outs, axis=0)

